# revision 17
# baseline (speedup 1.0000x reference)
import os
import sys
import numpy as np

sys.path.insert(0, "/opt/trn_rl_repo/concourse")
import bass
import tile
import mybir
from bass_utils import run_bass_kernel_spmd

# ---- environment workaround: walrus/bass codegen mismatch ----
# 1. default walrus rejects EVENT_SEMAPHORE_RANGE_CLEAR ("ISA wrong length")
#    -> use the b16 bazel build, which matches this bass's ISA tables.
# 2. walrus codegen supports only ONE embedded sync wait per instruction
#    -> split extra on_wait entries onto wait-only EventSemaphore preludes
#       (engines execute their queues in order, so this is equivalent).
_B16_WALRUS = (
    "/nix/store/wxap7svlj45h0lfm31d1axjjnzyl6qsy-b16-bazel-unstable-cc-"
    "2026-05-04-9a3fa1f3-rt-2026-05-04-ade39e0a/lib/python3.13/"
    "site-packages/neuronxcc/starfish/bin/walrus_driver"
)


def _fix_bir_file(path):
    import json
    with open(path) as f:
        bir = json.load(f)
    n_fixed = 0
    for fn in bir.get("functions", []):
        for blk in fn.get("blocks", []):
            out = []
            for inst in blk.get("instructions", []):
                si = inst.get("sync_info")
                ow = si.get("on_wait", []) if si else []
                if len(ow) > 1:
                    for j, w in enumerate(ow[:-1]):
                        out.append({
                            "debug": inst.get("debug", 0),
                            "engine": inst["engine"],
                            "ins": [],
                            "name": f"{inst['name']}-sw{j}",
                            "opcode": "EventSemaphore",
                            "outs": [],
                            "sync_info": {"on_update": [], "on_wait": [w]},
                        })
                    si["on_wait"] = [ow[-1]]
                    n_fixed += 1
                out.append(inst)
            blk["instructions"] = out
    if n_fixed:
        with open(path, "w") as f:
            json.dump(bir, f)
    return n_fixed


def _patch_walrus():
    from pathlib import Path
    mods = []
    try:
        import bass_utils as m1
        mods.append(m1)
    except ImportError:
        pass
    try:
        from concourse import bass_utils as m2
        mods.append(m2)
    except ImportError:
        pass
    seen = set()
    for mod in mods:
        if id(mod) in seen or getattr(mod, "_walrus_patched", False):
            continue
        seen.add(id(mod))
        orig = mod.bir_verify_and_optimise

        def patched(tmpdir, inp="bir.json", outp="file.neff", arch=None, *,
                    dve_root=None, _orig=orig):
            _fix_bir_file(str(Path(tmpdir) / inp))
            return _orig(tmpdir, inp, outp, arch, dve_root=dve_root)

        mod.bir_verify_and_optimise = patched
        mod.get_walrus_driver = lambda: _B16_WALRUS
        mod._walrus_patched = True


_patch_walrus()
# ---- end workaround ----

FP16 = np.float16
RSCALE = 2048.0
INV_RSCALE = float(1.0 / RSCALE)
N_CORES = 8
IPC = 16           # images per core
PASSES = 2         # image passes per core (SBUF capacity)
# input split-plane count per matmul layer
NPL = dict(c1=2, c2=2, c3=2, c4=2, c5=2, fc1=2)
BN_EPS = 1e-5

MIN = mybir.AluOpType.min
MAX = mybir.AluOpType.max
SUB = mybir.AluOpType.subtract
ADD = mybir.AluOpType.add
MULT = mybir.AluOpType.mult

LAST_RESULTS = None
LAST_EXEC_NS = None

# (name, wkey, ci_t, co_t, Hin, pool, tile_imgs)
CONV_LAYERS = [
    ("c2", "w2", 3, 6, 16, False, 2),
    ("c3", "w3", 6, 6, 16, True, 2),
    ("c4", "w4", 6, 12, 8, False, 8),
    ("c5", "w5", 12, 12, 8, True, 8),
]
NEXT = {"c1": "c2", "c2": "c3", "c3": "c4", "c4": "c5", "c5": "fc1"}


def _tern(w):
    import jax
    import jax.numpy as jnp
    cpu = jax.devices("cpu")[0]
    with jax.default_device(cpu):
        d = np.float32(0.7 * jnp.mean(jnp.abs(jnp.asarray(w))))
    return np.where(w > d, np.float32(1.0),
                    np.where(w < -d, np.float32(-1.0), np.float32(0.0))
                    ).astype(np.float32)


def _host_l0(x, w0, g, b, m, v):
    # x [B,3,32,32] f32 -> A0 [384,B,32,32] f32 (conv0 + ht + bn0 + celu)
    B = x.shape[0]
    h = x.transpose(1, 0, 2, 3)
    hp = np.pad(h, ((0, 0), (0, 0), (1, 1), (1, 1)))
    out = np.zeros((384, B, 32, 32), np.float32)
    for dy in range(3):
        for dx in range(3):
            patch = hp[:, :, dy:dy+32, dx:dx+32].reshape(3, -1)
            out += (w0[:, :, dy, dx] @ patch).reshape(384, B, 32, 32)
    out = np.clip(out, -1.0, 1.0)
    s = (g / np.sqrt(v + BN_EPS)).astype(np.float32)
    out = (out - m[:, None, None, None]) * s[:, None, None, None] \
        + b[:, None, None, None]
    out = np.where(out > 0, out,
                   np.expm1(np.minimum(out, 0.0))).astype(np.float32)
    return out


def _split2(a):
    p0 = a.astype(FP16)
    p1 = ((a - p0.astype(np.float32)) * np.float32(RSCALE)).astype(FP16)
    return p0, p1


def _wblob(q, ci_t, co_t):
    # q [Cout, Cin, 3, 3] -> [128, co_t, ci_t*9, 128] fp16
    # blob[ci_p, co, ci*9+dy*3+dx, co_p] = q[co*128+co_p, ci*128+ci_p, dy, dx]
    r = q.reshape(co_t, 128, ci_t, 128, 3, 3)
    return np.ascontiguousarray(
        r.transpose(3, 0, 2, 4, 5, 1).reshape(128, co_t, ci_t * 9, 128)
    ).astype(FP16)


def _build_nc():
    f16 = mybir.dt.float16
    f32 = mybir.dt.float32
    nc = bass.Bass("TRN2")
    a0d = nc.declare_dram_parameter(
        "a0", [128, IPC, 3, NPL["c1"], 34, 34], f16, isOutput=False)
    wds = {}
    for name, wkey, ci_t, co_t, _, _, _ in CONV_LAYERS:
        wds[wkey] = nc.declare_dram_parameter(
            wkey, [128, co_t, ci_t * 9, 128], f16, isOutput=False)
    w1d = nc.declare_dram_parameter("w1", [128, 3, 27, 128], f16,
                                    isOutput=False)
    fc1d = nc.declare_dram_parameter("fc1", [12, 128, 16, 1024], f16,
                                     isOutput=False)
    bn1m = nc.declare_dram_parameter("bn1m", [IPC, 1024], f32, isOutput=False)
    bn1s = nc.declare_dram_parameter("bn1s", [IPC, 1024], f32, isOutput=False)
    bn1b = nc.declare_dram_parameter("bn1b", [IPC, 1024], f32, isOutput=False)
    fc2d = nc.declare_dram_parameter("fc2", [IPC, 10, 1024], f32,
                                     isOutput=False)
    outd = nc.declare_dram_parameter("out", [IPC, 10], f32, isOutput=True)

    I = IPC // PASSES
    npl_fc = NPL["fc1"]

    from contextlib import ExitStack
    with tile.TileContext(nc) as tc:
        def write_planes(tp, src_ap, shape, dst_ap_fn, npl_out):
            # src_ap: fp32 conv/pool output; applies ht then split planes
            htv = tp.tile(shape, f32)
            nc.vector.tensor_scalar(htv[:], src_ap, 1.0, -1.0, MIN, MAX)
            nc.vector.tensor_copy(dst_ap_fn(0), htv[:])
            if npl_out == 2:
                resid = tp.tile(shape, f32)
                nc.vector.tensor_tensor(resid[:], htv[:], dst_ap_fn(0), SUB)
                nc.vector.tensor_scalar(dst_ap_fn(1), resid[:],
                                        float(RSCALE), None, MULT)

        def combine(tp, pss, shape, npl_in, will_pool=False):
            # DVE ops may read at most one PSUM input
            if npl_in == 2:
                tmp = tp.tile(shape, f32)
                nc.vector.tensor_scalar(tmp[:], pss[1][:], INV_RSCALE, None,
                                        MULT)
                conv = tp.tile(shape, f32)
                nc.vector.tensor_tensor(conv[:], tmp[:], pss[0][:], ADD)
                return conv[:]
            if will_pool:
                conv = tp.tile(shape, f32)
                nc.vector.tensor_copy(conv[:], pss[0][:])
                return conv[:]
            return pss[0][:]

        def pool2(tp, src_ap, shape):
            # shape: full conv-out shape [128, ..., H, W]; returns pooled AP
            H, W = shape[-2], shape[-1]
            csh = shape[:-1] + [W // 2]
            rsh = shape[:-2] + [H // 2, W // 2]
            ndim = len(shape)
            colsel0 = tuple([slice(None)] * (ndim - 1) + [slice(0, None, 2)])
            colsel1 = tuple([slice(None)] * (ndim - 1) + [slice(1, None, 2)])
            rowsel0 = tuple([slice(None)] * (ndim - 2)
                            + [slice(0, None, 2), slice(None)])
            rowsel1 = tuple([slice(None)] * (ndim - 2)
                            + [slice(1, None, 2), slice(None)])
            cm = tp.tile(csh, f32)
            nc.vector.tensor_tensor(cm[:], src_ap[colsel0], src_ap[colsel1],
                                    MAX)
            rm = tp.tile(rsh, f32)
            nc.vector.tensor_tensor(rm[:], cm[:][rowsel0], cm[:][rowsel1],
                                    MAX)
            return rm[:], rsh

        es_outer = ExitStack()
        a5p = es_outer.enter_context(tc.tile_pool(name="a5", bufs=1))
        a5t = a5p.tile([128, 12, npl_fc, IPC, 4, 4], f16)

        for p in range(PASSES):
            # ping-pong act pools: same-tag slot rotation reuses the buffer
            # of the layer-before-last (dead by then); LIFO close order holds.
            es_pass = ExitStack()
            pa_pool = es_pass.enter_context(
                tc.tile_pool(name=f"pa_{p}", bufs=1))
            pb_pool = es_pass.enter_context(
                tc.tile_pool(name=f"pb_{p}", bufs=1))

            # ---- conv1: stream a0 chunks ----
            npl1 = NPL["c1"]
            npl2 = NPL["c2"]
            a1t = pa_pool.tile([128, I, 3, npl2, 18, 18], f16, name="pa_t")
            for im in range(I):
                for ct in range(3):
                    nc.gpsimd.memset(a1t[:, im, ct], 0.0)
            with tc.tile_pool(name=f"a0_{p}", bufs=2) as a0p, \
                 tc.tile_pool(name=f"w1_{p}", bufs=1) as w1p, \
                 tc.tile_pool(name=f"ps1_{p}", bufs=2 * npl1,
                              space=bass.MemorySpace.PSUM) as pp1, \
                 tc.tile_pool(name=f"t1_{p}", bufs=2) as tp1:
                w1t = w1p.tile([128, 3, 27, 128], f16)
                nc.sync.dma_start(w1t[:], w1d[:])
                for i0 in range(0, I, 2):
                    a0t = a0p.tile([128, 2, 3, npl1, 34, 34], f16)
                    for j in range(2):
                        for ct in range(3):
                            nc.sync.dma_start(
                                a0t[:, j, ct],
                                a0d[:, p * I + i0 + j, ct])
                    for co in range(3):
                        for j in range(2):
                            for half in range(2):
                                pss = [pp1.tile([128, 16, 32], f32,
                                                name="pss")
                                       for _ in range(npl1)]
                                for pl in range(npl1):
                                    kk = 0
                                    for ci in range(3):
                                        for dy in range(3):
                                            for dx in range(3):
                                                r0 = half * 16 + dy
                                                rhs = a0t[:, j, ci, pl,
                                                          r0:r0+16,
                                                          dx:dx+32]
                                                nc.tensor.matmul(
                                                    pss[pl][:],
                                                    w1t[:, co, ci*9+dy*3+dx,
                                                        :],
                                                    rhs,
                                                    start=(kk == 0),
                                                    stop=(kk == 26))
                                                kk += 1
                                conv = combine(tp1, pss, [128, 16, 32], npl1)
                                pooled, psh = pool2(tp1, conv, [128, 16, 32])
                                im = i0 + j
                                rr = 1 + half * 8

                                def dst(pl, im=im, co=co, rr=rr):
                                    return a1t[:, im, co, pl, rr:rr+8, 1:17]
                                write_planes(tp1, pooled, psh, dst, npl2)

            # ---- conv2..conv5 ----
            src_t = a1t
            use_b = True
            for name, wkey, ci_t, co_t, Hin, pool, ti in CONV_LAYERS:
                npl_in = NPL[name]
                npl_out = NPL[NEXT[name]]
                last = (name == "c5")
                if not last:
                    Hp_out = (Hin // 2 if pool else Hin) + 2
                    dst_t = (pb_pool if use_b else pa_pool).tile(
                        [128, I, co_t, npl_out, Hp_out, Hp_out], f16,
                        name=("pb_t" if use_b else "pa_t"))
                    use_b = not use_b
                    for im in range(I):
                        for ct in range(co_t):
                            nc.gpsimd.memset(dst_t[:, im, ct], 0.0)
                with tc.tile_pool(name=f"w_{name}_{p}", bufs=2) as wp, \
                     tc.tile_pool(name=f"ps_{name}_{p}", bufs=2 * npl_in,
                                  space=bass.MemorySpace.PSUM) as pp, \
                     tc.tile_pool(name=f"t_{name}_{p}", bufs=2) as tp:
                    wdram = wds[wkey]
                    nk = ci_t * 9
                    for co in range(co_t):
                        wt = wp.tile([128, nk, 128], f16)
                        nc.sync.dma_start(wt[:], wdram[:, co])
                        for i0 in range(0, I, ti):
                            shape = [128, ti, Hin, Hin]
                            pss = [pp.tile(shape, f32, name="pss")
                                   for _ in range(npl_in)]
                            for pl in range(npl_in):
                                kk = 0
                                for ci in range(ci_t):
                                    for dy in range(3):
                                        for dx in range(3):
                                            rhs = src_t[:, i0:i0+ti, ci, pl,
                                                        dy:dy+Hin, dx:dx+Hin]
                                            nc.tensor.matmul(
                                                pss[pl][:],
                                                wt[:, ci*9+dy*3+dx, :],
                                                rhs,
                                                start=(kk == 0),
                                                stop=(kk == nk - 1))
                                            kk += 1
                            conv = combine(tp, pss, shape, npl_in)
                            if pool:
                                post, psh = pool2(tp, conv, shape)
                            else:
                                post, psh = conv, shape
                            if last:
                                def dst(pl, co=co, i0=i0):
                                    return a5t[:, co, pl,
                                               p * I + i0:p * I + i0 + ti,
                                               :, :]
                            else:
                                hh = psh[-1]

                                def dst(pl, co=co, i0=i0, hh=hh):
                                    return dst_t[:, i0:i0+ti, co, pl,
                                                 1:1+hh, 1:1+hh]
                            write_planes(tp, post, psh, dst, npl_out)
                if not last:
                    src_t = dst_t
            es_pass.close()

        # ---- fc1 / bn1 / ht / fc2 / log_softmax ----
        with tc.tile_pool(name="fcw", bufs=2) as fcw, \
             tc.tile_pool(name="fcps", bufs=1,
                          space=bass.MemorySpace.PSUM) as fcp, \
             tc.tile_pool(name="fct", bufs=1) as fct:
            psums = [[None, None] for _ in range(npl_fc)]
            for pl in range(npl_fc):
                for half in range(2):
                    psums[pl][half] = fcp.tile([IPC, 512], f32,
                                               name=f"fps{pl}{half}")
            for ct in range(12):
                wfc = fcw.tile([128, 16, 1024], f16)
                nc.sync.dma_start(wfc[:], fc1d[ct])
                for yx in range(16):
                    y, xx = yx // 4, yx % 4
                    for pl in range(npl_fc):
                        lhsT = a5t[:, ct, pl, :, y, xx]
                        for half in range(2):
                            nc.tensor.matmul(
                                psums[pl][half][:], lhsT,
                                wfc[:, yx, half*512:(half+1)*512],
                                start=(ct == 0 and yx == 0),
                                stop=(ct == 11 and yx == 15))
            f32_ = f32
            z = fct.tile([IPC, 1024], f32_)
            ztmp = fct.tile([IPC, 512], f32_)
            for half in range(2):
                zslice = z[:, half*512:(half+1)*512]
                if npl_fc == 2:
                    nc.vector.tensor_scalar(
                        ztmp[:], psums[1][half][:], INV_RSCALE, None, MULT)
                    nc.vector.tensor_tensor(
                        zslice, ztmp[:], psums[0][half][:], ADD)
                else:
                    nc.vector.tensor_copy(zslice, psums[0][half][:])
            bmt = fct.tile([IPC, 1024], f32_)
            bst = fct.tile([IPC, 1024], f32_)
            bbt = fct.tile([IPC, 1024], f32_)
            fc2t = fct.tile([IPC, 10, 1024], f32_)
            nc.sync.dma_start(bmt[:], bn1m[:])
            nc.sync.dma_start(bst[:], bn1s[:])
            nc.sync.dma_start(bbt[:], bn1b[:])
            nc.sync.dma_start(fc2t[:], fc2d[:])
            za = fct.tile([IPC, 1024], f32_)
            zb = fct.tile([IPC, 1024], f32_)
            nc.vector.tensor_tensor(za[:], z[:], bmt[:], SUB)
            nc.vector.tensor_tensor(zb[:], za[:], bst[:], MULT)
            nc.vector.tensor_tensor(za[:], zb[:], bbt[:], ADD)
            nc.vector.tensor_scalar(zb[:], za[:], 1.0, -1.0, MIN, MAX)
            dott = fct.tile([IPC, 1024], f32_)
            dot2 = fct.tile([IPC, 1024], f32_)
            z2 = fct.tile([IPC, 10], f32_)
            for i in range(10):
                nc.vector.tensor_tensor(dott[:], zb[:], fc2t[:, i, :], MULT)
                nc.scalar.activation(dot2[:], dott[:],
                                     mybir.ActivationFunctionType.Identity,
                                     accum_out=z2[:, i:i+1])
            mx = fct.tile([IPC, 1], f32_)
            nc.vector.tensor_reduce(mx[:], z2[:], axis=mybir.AxisListType.X,
                                    op=MAX)
            zc = fct.tile([IPC, 10], f32_)
            nc.vector.tensor_scalar(zc[:], z2[:], mx[:], None, SUB)
            ex = fct.tile([IPC, 10], f32_)
            ssum = fct.tile([IPC, 1], f32_)
            nc.scalar.activation(ex[:], zc[:],
                                 mybir.ActivationFunctionType.Exp,
                                 accum_out=ssum[:])
            lns = fct.tile([IPC, 1], f32_)
            nc.scalar.activation(lns[:], ssum[:],
                                 mybir.ActivationFunctionType.Ln)
            osb = fct.tile([IPC, 10], f32_)
            nc.vector.tensor_scalar(osb[:], zc[:], lns[:], None, SUB)
            nc.sync.dma_start(outd[:], osb[:])
        es_outer.close()
    return nc


def kernel(**inputs):
    global LAST_RESULTS
    inp = {k: np.asarray(v) for k, v in inputs.items()}
    x = inp["x"].astype(np.float32)
    B = x.shape[0]

    a0 = _host_l0(x, inp["w0"].astype(np.float32),
                  inp["bn0_g"].astype(np.float32),
                  inp["bn0_b"].astype(np.float32),
                  inp["bn0_m"].astype(np.float32),
                  inp["bn0_v"].astype(np.float32))

    # split planes + pad + per-core layout [128, img, 3, npl, 34, 34]
    pl0, pl1 = _split2(a0)
    A = np.stack([pl0, pl1], axis=0)          # [2, 384, B, 32, 32]
    A = A.reshape(2, 3, 128, B, 32, 32).transpose(2, 3, 1, 0, 4, 5)
    Ap = np.zeros((128, B, 3, NPL["c1"], 34, 34), FP16)
    Ap[:, :, :, :, 1:33, 1:33] = A

    q = {k: _tern(inp[k].astype(np.float32))
         for k in ["w1", "w2", "w3", "w4", "w5", "fc1_w", "fc2_w"]}
    w1b = _wblob(q["w1"], 3, 3)
    w2b = _wblob(q["w2"], 3, 6)
    w3b = _wblob(q["w3"], 6, 6)
    w4b = _wblob(q["w4"], 6, 12)
    w5b = _wblob(q["w5"], 12, 12)
    fc1b = np.ascontiguousarray(
        q["fc1_w"].reshape(1024, 12, 128, 16).transpose(1, 2, 3, 0)
    ).astype(FP16)
    s1 = (inp["bn1_g"] / np.sqrt(inp["bn1_v"] + BN_EPS)).astype(np.float32)
    bm = np.ascontiguousarray(
        np.broadcast_to(inp["bn1_m"].astype(np.float32), (IPC, 1024)))
    bs = np.ascontiguousarray(np.broadcast_to(s1, (IPC, 1024)))
    bb = np.ascontiguousarray(
        np.broadcast_to(inp["bn1_b"].astype(np.float32), (IPC, 1024)))
    fc2b = np.ascontiguousarray(
        np.broadcast_to(q["fc2_w"][None], (IPC, 10, 1024))).astype(np.float32)

    nc = _build_nc()
    common = dict(w1=w1b, w2=w2b, w3=w3b, w4=w4b, w5=w5b, fc1=fc1b,
                  bn1m=bm, bn1s=bs, bn1b=bb, fc2=fc2b)
    in_maps = []
    for c in range(N_CORES):
        m = dict(common)
        m["a0"] = np.ascontiguousarray(Ap[:, c*IPC:(c+1)*IPC])
        in_maps.append(m)
    import time
    t0 = time.perf_counter()
    res = run_bass_kernel_spmd(nc, in_maps, core_ids=list(range(N_CORES)))
    t1 = time.perf_counter()
    LAST_RESULTS = res
    global LAST_EXEC_NS
    LAST_EXEC_NS = (t1 - t0) * 1e9
    out = np.concatenate(
        [np.asarray(res.results[c]["out"]) for c in range(N_CORES)], axis=0)
    return out.astype(np.float32)


# revision 19
# speedup vs baseline: 1.3784x; 1.3784x over previous
import os
import sys
import numpy as np

sys.path.insert(0, "/opt/trn_rl_repo/concourse")
import bass
import tile
import mybir
from bass_utils import run_bass_kernel_spmd

# ---- environment workaround: walrus/bass codegen mismatch ----
# 1. default walrus rejects EVENT_SEMAPHORE_RANGE_CLEAR ("ISA wrong length")
#    -> use the b16 bazel build, which matches this bass's ISA tables.
# 2. walrus codegen supports only ONE embedded sync wait per instruction
#    -> split extra on_wait entries onto wait-only EventSemaphore preludes
#       (engines execute their queues in order, so this is equivalent).
_B16_WALRUS = (
    "/nix/store/wxap7svlj45h0lfm31d1axjjnzyl6qsy-b16-bazel-unstable-cc-"
    "2026-05-04-9a3fa1f3-rt-2026-05-04-ade39e0a/lib/python3.13/"
    "site-packages/neuronxcc/starfish/bin/walrus_driver"
)


def _fix_bir_file(path):
    import json
    with open(path) as f:
        bir = json.load(f)
    n_fixed = 0
    for fn in bir.get("functions", []):
        for blk in fn.get("blocks", []):
            out = []
            for inst in blk.get("instructions", []):
                si = inst.get("sync_info")
                ow = si.get("on_wait", []) if si else []
                if len(ow) > 1:
                    for j, w in enumerate(ow[:-1]):
                        out.append({
                            "debug": inst.get("debug", 0),
                            "engine": inst["engine"],
                            "ins": [],
                            "name": f"{inst['name']}-sw{j}",
                            "opcode": "EventSemaphore",
                            "outs": [],
                            "sync_info": {"on_update": [], "on_wait": [w]},
                        })
                    si["on_wait"] = [ow[-1]]
                    n_fixed += 1
                out.append(inst)
            blk["instructions"] = out
    if n_fixed:
        with open(path, "w") as f:
            json.dump(bir, f)
    return n_fixed


def _patch_walrus():
    from pathlib import Path
    mods = []
    try:
        import bass_utils as m1
        mods.append(m1)
    except ImportError:
        pass
    try:
        from concourse import bass_utils as m2
        mods.append(m2)
    except ImportError:
        pass
    seen = set()
    for mod in mods:
        if id(mod) in seen or getattr(mod, "_walrus_patched", False):
            continue
        seen.add(id(mod))
        orig = mod.bir_verify_and_optimise

        def patched(tmpdir, inp="bir.json", outp="file.neff", arch=None, *,
                    dve_root=None, _orig=orig):
            _fix_bir_file(str(Path(tmpdir) / inp))
            return _orig(tmpdir, inp, outp, arch, dve_root=dve_root)

        mod.bir_verify_and_optimise = patched
        mod.get_walrus_driver = lambda: _B16_WALRUS
        mod._walrus_patched = True


_patch_walrus()
# ---- end workaround ----

FP16 = np.float16
RSCALE = 2048.0
INV_RSCALE = float(1.0 / RSCALE)
N_CORES = 8
IPC = 16           # images per core
PASSES = 2         # image passes per core (SBUF capacity)
# input split-plane count per matmul layer
NPL = dict(c1=2, c2=2, c3=2, c4=2, c5=2, fc1=2)
BN_EPS = 1e-5

MIN = mybir.AluOpType.min
MAX = mybir.AluOpType.max
SUB = mybir.AluOpType.subtract
ADD = mybir.AluOpType.add
MULT = mybir.AluOpType.mult

LAST_RESULTS = None
LAST_EXEC_NS = None

# (name, wkey, ci_t, co_t, Hin, pool, tile_imgs)
CONV_LAYERS = [
    ("c2", "w2", 3, 6, 16, False, 2),
    ("c3", "w3", 6, 6, 16, True, 2),
    ("c4", "w4", 6, 12, 8, False, 8),
    ("c5", "w5", 12, 12, 8, True, 8),
]
NEXT = {"c1": "c2", "c2": "c3", "c3": "c4", "c4": "c5", "c5": "fc1"}


def _tern(w):
    import jax
    import jax.numpy as jnp
    cpu = jax.devices("cpu")[0]
    with jax.default_device(cpu):
        d = np.float32(0.7 * jnp.mean(jnp.abs(jnp.asarray(w))))
    return np.where(w > d, np.float32(1.0),
                    np.where(w < -d, np.float32(-1.0), np.float32(0.0))
                    ).astype(np.float32)


def _host_l0(x, w0, g, b, m, v):
    # x [B,3,32,32] f32 -> A0 [384,B,32,32] f32 (conv0 + ht + bn0 + celu)
    B = x.shape[0]
    h = x.transpose(1, 0, 2, 3)
    hp = np.pad(h, ((0, 0), (0, 0), (1, 1), (1, 1)))
    out = np.zeros((384, B, 32, 32), np.float32)
    for dy in range(3):
        for dx in range(3):
            patch = hp[:, :, dy:dy+32, dx:dx+32].reshape(3, -1)
            out += (w0[:, :, dy, dx] @ patch).reshape(384, B, 32, 32)
    out = np.clip(out, -1.0, 1.0)
    s = (g / np.sqrt(v + BN_EPS)).astype(np.float32)
    out = (out - m[:, None, None, None]) * s[:, None, None, None] \
        + b[:, None, None, None]
    out = np.where(out > 0, out,
                   np.expm1(np.minimum(out, 0.0))).astype(np.float32)
    return out


def _split2(a):
    p0 = a.astype(FP16)
    p1 = ((a - p0.astype(np.float32)) * np.float32(RSCALE)).astype(FP16)
    return p0, p1


def _wblob(q, ci_t, co_t):
    # q [Cout, Cin, 3, 3] -> [128, co_t, ci_t*9, 128] fp16
    # blob[ci_p, co, ci*9+dy*3+dx, co_p] = q[co*128+co_p, ci*128+ci_p, dy, dx]
    r = q.reshape(co_t, 128, ci_t, 128, 3, 3)
    return np.ascontiguousarray(
        r.transpose(3, 0, 2, 4, 5, 1).reshape(128, co_t, ci_t * 9, 128)
    ).astype(FP16)


def _build_nc():
    f16 = mybir.dt.float16
    f32 = mybir.dt.float32
    nc = bass.Bass("TRN2")
    a0d = nc.declare_dram_parameter(
        "a0", [128, IPC, 3, NPL["c1"], 34, 34], f16, isOutput=False)
    wds = {}
    for name, wkey, ci_t, co_t, _, _, _ in CONV_LAYERS:
        wds[wkey] = nc.declare_dram_parameter(
            wkey, [128, co_t, ci_t * 9, 128], f16, isOutput=False)
    w1d = nc.declare_dram_parameter("w1", [128, 3, 27, 128], f16,
                                    isOutput=False)
    fc1d = nc.declare_dram_parameter("fc1", [12, 128, 16, 1024], f16,
                                     isOutput=False)
    bn1m = nc.declare_dram_parameter("bn1m", [IPC, 1024], f32, isOutput=False)
    bn1s = nc.declare_dram_parameter("bn1s", [IPC, 1024], f32, isOutput=False)
    bn1b = nc.declare_dram_parameter("bn1b", [IPC, 1024], f32, isOutput=False)
    fc2d = nc.declare_dram_parameter("fc2", [IPC, 10, 1024], f32,
                                     isOutput=False)
    outd = nc.declare_dram_parameter("out", [IPC, 10], f32, isOutput=True)

    I = IPC // PASSES
    npl_fc = NPL["fc1"]

    from contextlib import ExitStack
    with tile.TileContext(nc) as tc:
        def write_planes(tp, src_ap, shape, dst_ap_fn, npl_out):
            # src_ap: fp32 conv/pool output; applies ht then split planes
            htv = tp.tile(shape, f32)
            nc.vector.tensor_scalar(htv[:], src_ap, 1.0, -1.0, MIN, MAX)
            nc.vector.tensor_copy(dst_ap_fn(0), htv[:])
            if npl_out == 2:
                resid = tp.tile(shape, f32)
                nc.vector.tensor_tensor(resid[:], htv[:], dst_ap_fn(0), SUB)
                nc.vector.tensor_scalar(dst_ap_fn(1), resid[:],
                                        float(RSCALE), None, MULT)

        def combine(tp, pss, shape, npl_in, will_pool=False):
            # DVE ops may read at most one PSUM input
            if npl_in == 2:
                tmp = tp.tile(shape, f32)
                nc.vector.tensor_scalar(tmp[:], pss[1][:], INV_RSCALE, None,
                                        MULT)
                conv = tp.tile(shape, f32)
                nc.vector.tensor_tensor(conv[:], tmp[:], pss[0][:], ADD)
                return conv[:]
            if will_pool:
                conv = tp.tile(shape, f32)
                nc.vector.tensor_copy(conv[:], pss[0][:])
                return conv[:]
            return pss[0][:]

        def pool2(tp, src_ap, shape):
            # shape: full conv-out shape [128, ..., H, W]; returns pooled AP
            H, W = shape[-2], shape[-1]
            csh = shape[:-1] + [W // 2]
            rsh = shape[:-2] + [H // 2, W // 2]
            ndim = len(shape)
            colsel0 = tuple([slice(None)] * (ndim - 1) + [slice(0, None, 2)])
            colsel1 = tuple([slice(None)] * (ndim - 1) + [slice(1, None, 2)])
            rowsel0 = tuple([slice(None)] * (ndim - 2)
                            + [slice(0, None, 2), slice(None)])
            rowsel1 = tuple([slice(None)] * (ndim - 2)
                            + [slice(1, None, 2), slice(None)])
            cm = tp.tile(csh, f32)
            nc.vector.tensor_tensor(cm[:], src_ap[colsel0], src_ap[colsel1],
                                    MAX)
            rm = tp.tile(rsh, f32)
            nc.vector.tensor_tensor(rm[:], cm[:][rowsel0], cm[:][rowsel1],
                                    MAX)
            return rm[:], rsh

        es_outer = ExitStack()
        a5p = es_outer.enter_context(tc.tile_pool(name="a5", bufs=1))
        a5t = a5p.tile([128, 12, npl_fc, IPC, 4, 4], f16)

        for p in range(PASSES):
            # ping-pong act pools: same-tag slot rotation reuses the buffer
            # of the layer-before-last (dead by then); LIFO close order holds.
            es_pass = ExitStack()
            pa_pool = es_pass.enter_context(
                tc.tile_pool(name=f"pa_{p}", bufs=1))
            pb_pool = es_pass.enter_context(
                tc.tile_pool(name=f"pb_{p}", bufs=1))

            # ---- conv1: stream a0 chunks ----
            npl1 = NPL["c1"]
            npl2 = NPL["c2"]
            a1t = pa_pool.tile([128, I, 3, npl2, 18, 18], f16, name="pa_t")
            for im in range(I):
                for ct in range(3):
                    nc.gpsimd.memset(a1t[:, im, ct], 0.0)
            with tc.tile_pool(name=f"a0_{p}", bufs=2) as a0p, \
                 tc.tile_pool(name=f"w1_{p}", bufs=1) as w1p, \
                 tc.tile_pool(name=f"ps1_{p}", bufs=2 * npl1,
                              space=bass.MemorySpace.PSUM) as pp1, \
                 tc.tile_pool(name=f"t1_{p}", bufs=2) as tp1:
                w1t = w1p.tile([128, 3, 27, 128], f16)
                nc.sync.dma_start(w1t[:], w1d[:])
                for i0 in range(0, I, 2):
                    a0t = a0p.tile([128, 2, 3, npl1, 34, 34], f16)
                    for j in range(2):
                        for ct in range(3):
                            nc.sync.dma_start(
                                a0t[:, j, ct],
                                a0d[:, p * I + i0 + j, ct])
                    for co in range(3):
                        for j in range(2):
                            for half in range(2):
                                pss = [pp1.tile([128, 16, 32], f32,
                                                name="pss")
                                       for _ in range(npl1)]
                                for pl in range(npl1):
                                    kk = 0
                                    for ci in range(3):
                                        for dy in range(3):
                                            for dx in range(3):
                                                r0 = half * 16 + dy
                                                rhs = a0t[:, j, ci, pl,
                                                          r0:r0+16,
                                                          dx:dx+32]
                                                nc.tensor.matmul(
                                                    pss[pl][:],
                                                    w1t[:, co, ci*9+dy*3+dx,
                                                        :],
                                                    rhs,
                                                    start=(kk == 0),
                                                    stop=(kk == 26))
                                                kk += 1
                                conv = combine(tp1, pss, [128, 16, 32], npl1)
                                pooled, psh = pool2(tp1, conv, [128, 16, 32])
                                im = i0 + j
                                rr = 1 + half * 8

                                def dst(pl, im=im, co=co, rr=rr):
                                    return a1t[:, im, co, pl, rr:rr+8, 1:17]
                                write_planes(tp1, pooled, psh, dst, npl2)

            # ---- conv2..conv5 ----
            src_t = a1t
            use_b = True
            for name, wkey, ci_t, co_t, Hin, pool, ti in CONV_LAYERS:
                npl_in = NPL[name]
                npl_out = NPL[NEXT[name]]
                last = (name == "c5")
                if not last:
                    Hp_out = (Hin // 2 if pool else Hin) + 2
                    dst_t = (pb_pool if use_b else pa_pool).tile(
                        [128, I, co_t, npl_out, Hp_out, Hp_out], f16,
                        name=("pb_t" if use_b else "pa_t"))
                    use_b = not use_b
                    for im in range(I):
                        for ct in range(co_t):
                            nc.gpsimd.memset(dst_t[:, im, ct], 0.0)
                with tc.tile_pool(name=f"w_{name}_{p}", bufs=2) as wp, \
                     tc.tile_pool(name=f"ps_{name}_{p}", bufs=2 * npl_in,
                                  space=bass.MemorySpace.PSUM) as pp, \
                     tc.tile_pool(name=f"t_{name}_{p}", bufs=2) as tp:
                    wdram = wds[wkey]
                    nk = ci_t * 9
                    for co in range(co_t):
                        wt = wp.tile([128, nk, 128], f16)
                        nc.sync.dma_start(wt[:], wdram[:, co])
                        for i0 in range(0, I, ti):
                            shape = [128, ti, Hin, Hin]
                            pss = [pp.tile(shape, f32, name="pss")
                                   for _ in range(npl_in)]
                            for pl in range(npl_in):
                                kk = 0
                                for ci in range(ci_t):
                                    for dy in range(3):
                                        for dx in range(3):
                                            rhs = src_t[:, i0:i0+ti, ci, pl,
                                                        dy:dy+Hin, dx:dx+Hin]
                                            nc.tensor.matmul(
                                                pss[pl][:],
                                                wt[:, ci*9+dy*3+dx, :],
                                                rhs,
                                                start=(kk == 0),
                                                stop=(kk == nk - 1))
                                            kk += 1
                            conv = combine(tp, pss, shape, npl_in)
                            if pool:
                                post, psh = pool2(tp, conv, shape)
                            else:
                                post, psh = conv, shape
                            if last:
                                def dst(pl, co=co, i0=i0):
                                    return a5t[:, co, pl,
                                               p * I + i0:p * I + i0 + ti,
                                               :, :]
                            else:
                                hh = psh[-1]

                                def dst(pl, co=co, i0=i0, hh=hh):
                                    return dst_t[:, i0:i0+ti, co, pl,
                                                 1:1+hh, 1:1+hh]
                            write_planes(tp, post, psh, dst, npl_out)
                if not last:
                    src_t = dst_t
            es_pass.close()

        # ---- fc1 / bn1 / ht / fc2 / log_softmax ----
        with tc.tile_pool(name="fcw", bufs=2) as fcw, \
             tc.tile_pool(name="fcps", bufs=1,
                          space=bass.MemorySpace.PSUM) as fcp, \
             tc.tile_pool(name="fct", bufs=1) as fct:
            psums = [[None, None] for _ in range(npl_fc)]
            for pl in range(npl_fc):
                for half in range(2):
                    psums[pl][half] = fcp.tile([IPC, 512], f32,
                                               name=f"fps{pl}{half}")
            for ct in range(12):
                wfc = fcw.tile([128, 16, 1024], f16)
                nc.sync.dma_start(wfc[:], fc1d[ct])
                for yx in range(16):
                    y, xx = yx // 4, yx % 4
                    for pl in range(npl_fc):
                        lhsT = a5t[:, ct, pl, :, y, xx]
                        for half in range(2):
                            nc.tensor.matmul(
                                psums[pl][half][:], lhsT,
                                wfc[:, yx, half*512:(half+1)*512],
                                start=(ct == 0 and yx == 0),
                                stop=(ct == 11 and yx == 15))
            f32_ = f32
            z = fct.tile([IPC, 1024], f32_)
            ztmp = fct.tile([IPC, 512], f32_)
            for half in range(2):
                zslice = z[:, half*512:(half+1)*512]
                if npl_fc == 2:
                    nc.vector.tensor_scalar(
                        ztmp[:], psums[1][half][:], INV_RSCALE, None, MULT)
                    nc.vector.tensor_tensor(
                        zslice, ztmp[:], psums[0][half][:], ADD)
                else:
                    nc.vector.tensor_copy(zslice, psums[0][half][:])
            bmt = fct.tile([IPC, 1024], f32_)
            bst = fct.tile([IPC, 1024], f32_)
            bbt = fct.tile([IPC, 1024], f32_)
            fc2t = fct.tile([IPC, 10, 1024], f32_)
            nc.sync.dma_start(bmt[:], bn1m[:])
            nc.sync.dma_start(bst[:], bn1s[:])
            nc.sync.dma_start(bbt[:], bn1b[:])
            nc.sync.dma_start(fc2t[:], fc2d[:])
            za = fct.tile([IPC, 1024], f32_)
            zb = fct.tile([IPC, 1024], f32_)
            nc.vector.tensor_tensor(za[:], z[:], bmt[:], SUB)
            nc.vector.tensor_tensor(zb[:], za[:], bst[:], MULT)
            nc.vector.tensor_tensor(za[:], zb[:], bbt[:], ADD)
            nc.vector.tensor_scalar(zb[:], za[:], 1.0, -1.0, MIN, MAX)
            dott = fct.tile([IPC, 1024], f32_)
            dot2 = fct.tile([IPC, 1024], f32_)
            z2 = fct.tile([IPC, 10], f32_)
            for i in range(10):
                nc.vector.tensor_tensor(dott[:], zb[:], fc2t[:, i, :], MULT)
                nc.scalar.activation(dot2[:], dott[:],
                                     mybir.ActivationFunctionType.Identity,
                                     accum_out=z2[:, i:i+1])
            mx = fct.tile([IPC, 1], f32_)
            nc.vector.tensor_reduce(mx[:], z2[:], axis=mybir.AxisListType.X,
                                    op=MAX)
            zc = fct.tile([IPC, 10], f32_)
            nc.vector.tensor_scalar(zc[:], z2[:], mx[:], None, SUB)
            ex = fct.tile([IPC, 10], f32_)
            ssum = fct.tile([IPC, 1], f32_)
            nc.scalar.activation(ex[:], zc[:],
                                 mybir.ActivationFunctionType.Exp,
                                 accum_out=ssum[:])
            lns = fct.tile([IPC, 1], f32_)
            nc.scalar.activation(lns[:], ssum[:],
                                 mybir.ActivationFunctionType.Ln)
            osb = fct.tile([IPC, 10], f32_)
            nc.vector.tensor_scalar(osb[:], zc[:], lns[:], None, SUB)
            nc.sync.dma_start(outd[:], osb[:])
        es_outer.close()
    return nc


def kernel(**inputs):
    global LAST_RESULTS
    inp = {k: np.asarray(v) for k, v in inputs.items()}
    x = inp["x"].astype(np.float32)
    B = x.shape[0]

    a0 = _host_l0(x, inp["w0"].astype(np.float32),
                  inp["bn0_g"].astype(np.float32),
                  inp["bn0_b"].astype(np.float32),
                  inp["bn0_m"].astype(np.float32),
                  inp["bn0_v"].astype(np.float32))

    # split planes + pad + per-core layout [128, img, 3, npl, 34, 34]
    pl0, pl1 = _split2(a0)
    A = np.stack([pl0, pl1], axis=0)          # [2, 384, B, 32, 32]
    A = A.reshape(2, 3, 128, B, 32, 32).transpose(2, 3, 1, 0, 4, 5)
    Ap = np.zeros((128, B, 3, NPL["c1"], 34, 34), FP16)
    Ap[:, :, :, :, 1:33, 1:33] = A

    q = {k: _tern(inp[k].astype(np.float32))
         for k in ["w1", "w2", "w3", "w4", "w5", "fc1_w", "fc2_w"]}
    w1b = _wblob(q["w1"], 3, 3)
    w2b = _wblob(q["w2"], 3, 6)
    w3b = _wblob(q["w3"], 6, 6)
    w4b = _wblob(q["w4"], 6, 12)
    w5b = _wblob(q["w5"], 12, 12)
    fc1b = np.ascontiguousarray(
        q["fc1_w"].reshape(1024, 12, 128, 16).transpose(1, 2, 3, 0)
    ).astype(FP16)
    s1 = (inp["bn1_g"] / np.sqrt(inp["bn1_v"] + BN_EPS)).astype(np.float32)
    bm = np.ascontiguousarray(
        np.broadcast_to(inp["bn1_m"].astype(np.float32), (IPC, 1024)))
    bs = np.ascontiguousarray(np.broadcast_to(s1, (IPC, 1024)))
    bb = np.ascontiguousarray(
        np.broadcast_to(inp["bn1_b"].astype(np.float32), (IPC, 1024)))
    fc2b = np.ascontiguousarray(
        np.broadcast_to(q["fc2_w"][None], (IPC, 10, 1024))).astype(np.float32)

    nc = _build_nc()
    common = dict(w1=w1b, w2=w2b, w3=w3b, w4=w4b, w5=w5b, fc1=fc1b,
                  bn1m=bm, bn1s=bs, bn1b=bb, fc2=fc2b)
    in_maps = []
    for c in range(N_CORES):
        m = dict(common)
        m["a0"] = np.ascontiguousarray(Ap[:, c*IPC:(c+1)*IPC])
        in_maps.append(m)
    import time
    t0 = time.perf_counter()
    res = run_bass_kernel_spmd(nc, in_maps, core_ids=list(range(N_CORES)))
    t1 = time.perf_counter()
    LAST_RESULTS = res
    global LAST_EXEC_NS
    LAST_EXEC_NS = (t1 - t0) * 1e9
    out = np.concatenate(
        [np.asarray(res.results[c]["out"]) for c in range(N_CORES)], axis=0)
    return out.astype(np.float32)
